# revision 14
# baseline (speedup 1.0000x reference)
"""Trainium2 Bass kernel for CapsuleBlock (dynamic routing).

Reference computation:
  hats[b,n,k,o] = sum_d x[b,n,d] * W[n,k,d,o]       x:[64,2048,8] W:[2048,16,8,16]
  3 routing iterations (softmax over k, weighted sum over n, squash over o)
  output: [64, 16, 16]

Sharding: data-parallel over batch B=64 across 8 cores (B_local=8), W replicated.

Per-core layout notation: n = g*16 + v  (g in [0,128) "group", v in [0,16)),
d in [0,8), k in [0,16) out-capsule, o in [0,16) out-dim.

Key trick: einsum runs on the tensor engine with a block-diagonal lhsT
  lhsT[(v,d), (b,v')] = x[b, g*16+v', d] * delta_{v,v'}
so one 128-wide matmul per group computes hats for 16 n's x 8 b's at once:
  psum[(b,v'), (k,o)] = sum_{(v,d)} lhsT * W[g*16+v, k, d, o]
H lives in SBUF as [p=(b,v'), f=(g, k, o)] in bf16 (8MB).

W streaming: W arrives [n,k,d,o]; the matmul needs [(v,d),(k,o)] per group.
Loading that layout directly from HBM costs 1024 DMAs of 64B runs (~10ms).
Instead: 16 x 1MB contiguous loads [n-batch, (k,d,o)], an on-chip permute
copy to [(n),(d,k,o)] (which also rounds to f32r for the 1-cycle/col PE
path), then one SBUF->SBUF DMA per group (SBUF-SBUF has no small-run
penalty) into [(v,d),(k,o)].

s0 (uniform-c iteration): a second accumulating matmul per group with the
same operands gives psum_s0[(b,v'),(k,o)] = sum_g hats; a small ONESB16
matmul then averages over v'.

Routing iters 1,2 sweep H per 16-group chunk: bf16 product vs broadcast
OutB + reduce over o -> bias += a; softmax over k; c-lhsT built with 8
per-b indicator tensor_scalar ops; 16 accumulating bf16 matmuls into
psum_R1[(b,k'), (k,o)]; diagonal extract (k==k' mask) -> squash -> out.

Toolchain workarounds (this container):
- walrus codegen allows only 1 sync wait/instruction -> run Bacc's
  move_matmul_waits_to_ldweights + generate_event_semaphores passes.
- gpsimd int shift/mod ops and DVE tensor_tensor_reduce hit "ISA wrong
  length" -> constant masks precomputed on host, squash unfused.
- DMA source APs that split the partition dim read wrong partitions ->
  keep rearranges on the DRAM side of every DMA.
"""

import numpy as np

import concourse.bass as bass
import concourse.mybir as mybir
import concourse.tile as tile
from concourse.bass import ds
from concourse.bass_utils import run_bass_kernel_spmd

F32 = mybir.dt.float32
F32R = mybir.dt.float32r
BF16 = mybir.dt.bfloat16
AX = mybir.AxisListType
OP = mybir.AluOpType
ACT_F = mybir.ActivationFunctionType

# per-core problem dims
B = 8        # local batch (64 / 8 cores)
N = 2048     # input capsules
K = 16       # output capsules
O = 16       # output capsule dim
D = 8        # input capsule dim
V = 16       # n's per group
G = N // V   # 128 groups
GL = 8       # groups per W batch
NB = G // GL  # 16 W batches (128 n's each)
P = 128
KO = K * O   # 256

NUM_ROUTINGS = 3
CHUNK = 16   # groups per sweep chunk


# host-precomputed constant masks, packed as one [128, CF] f32 input
# (gpsimd int shift/mod ops hit an "ISA wrong length" walrus codegen bug in
# this toolchain, so the masks cannot be built on-device)
C_IDENT = 0            # [128, 128] identity (PE transpose)
C_M = 128              # [128, B*V]  M[(v,d),(b,v')] = delta_{v,v'}
C_IB = C_M + B * V     # [128, B]    Ib[(b,v'),b'] = delta_{b,b'}
C_IB16 = C_IB + B      # [128, B]    Ib / 16
C_MK = C_IB16 + B      # [128, KO]   MK[(b,k'),(k,o)] = delta_{k,k'}
CF = C_MK + KO


def _build_consts() -> np.ndarray:
    c = np.zeros((P, CF), dtype=np.float32)
    p = np.arange(P)
    c[:, C_IDENT:C_IDENT + P] = np.eye(P, dtype=np.float32)
    # M: partition p=(v,d) with v=p>>3 ; free f=(b,v') with v'=f%16
    f = np.arange(B * V)
    c[:, C_M:C_M + B * V] = ((p >> 3)[:, None] == (f % V)[None, :])
    # Ib: partition p=(b,v') with b=p>>4 ; free b'
    fb = np.arange(B)
    c[:, C_IB:C_IB + B] = ((p >> 4)[:, None] == fb[None, :])
    c[:, C_IB16:C_IB16 + B] = c[:, C_IB:C_IB + B] / V
    # MK: partition p=(b,k') with k'=p%16 ; free f=(k,o) with k=f//16
    fk = np.arange(KO)
    c[:, C_MK:C_MK + KO] = ((p % K)[:, None] == (fk // O)[None, :])
    return c


def build_kernel(reps=1):
    nc = bass.Bass(trn_type="TRN2")

    x_d = nc.dram_tensor("x", [B, N, D], F32, kind="ExternalInput")
    w_d = nc.dram_tensor("w", [N, K, D, O], F32, kind="ExternalInput")
    c_d = nc.dram_tensor("consts", [P, CF], F32, kind="ExternalInput")
    out_d = nc.dram_tensor("out", [B, K, O], F32, kind="ExternalOutput")
    # scratch for broadcasting the per-iteration output back to SBUF layouts
    scr = nc.dram_tensor("scr", [B, K, O], F32, kind="Internal")

    with tile.TileContext(nc) as tc, nc.allow_low_precision(
            reason="bf16/f32r capsule routing, validated vs fp32 reference"):
        for _ in range(reps):
            _capsule(tc, x_d, w_d, c_d, out_d, scr)

    # TRN2 walrus codegen only allows one sync wait per instruction; these
    # Bacc passes split multi-wait instructions via event semaphores.
    import bass_rust as _bass_rust
    _bass_rust.move_matmul_waits_to_ldweights(nc.m)
    _bass_rust.generate_event_semaphores(nc)
    return nc


def _capsule(tc, x_d, w_d, c_d, out_d, scr):
    nc = tc.nc

    from contextlib import ExitStack
    ctx = ExitStack()
    consts = ctx.enter_context(tc.tile_pool(name="consts", bufs=1))
    hpool = ctx.enter_context(tc.tile_pool(name="hpool", bufs=1))
    wnatp = ctx.enter_context(tc.tile_pool(name="wnatp", bufs=2))
    wpermp = ctx.enter_context(tc.tile_pool(name="wpermp", bufs=2))
    wtp = ctx.enter_context(tc.tile_pool(name="wtp", bufs=2))
    ltp = ctx.enter_context(tc.tile_pool(name="ltp", bufs=2))
    small = ctx.enter_context(tc.tile_pool(name="small", bufs=2))
    sweep = ctx.enter_context(tc.tile_pool(name="sweep", bufs=2))
    psum_e = ctx.enter_context(tc.tile_pool(name="psum_e", bufs=3, space="PSUM"))
    psum_s = ctx.enter_context(tc.tile_pool(name="psum_s", bufs=1, space="PSUM"))
    psum_r = ctx.enter_context(tc.tile_pool(name="psum_r", bufs=1, space="PSUM"))

    # ---------------- constants (host-precomputed, one DMA) ----------------
    CON = consts.tile([P, CF], F32)
    nc.sync.dma_start(CON, c_d[:])
    ident = CON[:, ds(C_IDENT, P)]
    M = CON[:, ds(C_M, B * V)].rearrange("p (b v) -> p b v", b=B)
    Ib = CON[:, ds(C_IB, B)]
    ONESB16 = CON[:, ds(C_IB16, B)]
    MK = CON[:, ds(C_MK, KO)]

    # ---------------- load + transpose x ----------------
    # x flat index = b*16384 + n*8 + d with n = nb*128 + gi*16 + v:
    #   X1[p=(b, nb), f=(gi, v, d)]
    XT2 = consts.tile([P, NB, GL, B], F32)  # XT2[(v,d), nb, gi, b]
    with tc.tile_pool(name="xprep", bufs=1) as xprep, \
         tc.tile_pool(name="psum_t", bufs=2, space="PSUM") as psum_t:
        X1 = xprep.tile([P, GL, V * D], F32)
        nc.sync.dma_start(X1, x_d.rearrange("b (nb gi v) d -> (b nb) gi (v d)",
                                            nb=NB, gi=GL, v=V))
        for gi in range(GL):
            pt = psum_t.tile([P, P], F32)
            nc.tensor.transpose(pt, X1[:, gi], ident)
            nc.vector.tensor_copy(
                XT2[:, :, gi, :], pt.rearrange("p (b nb) -> p nb b", b=B))

    # ---------------- einsum: H[(b,v'), (g,k,o)] bf16 ----------------
    H = hpool.tile([P, G, KO], BF16)
    ps0 = psum_s.tile([P, KO], F32, name="ps0")  # sum_g hats accumulator

    for nb in range(NB):
        # natural-layout W batch: 1MB contiguous
        wnat = wnatp.tile([P, K * D * O], F32, tag="wnat")
        nc.sync.dma_start(
            wnat, w_d[ds(nb * P, P)].rearrange("n k d o -> n (k d o)"))
        # permute (k,d,o)->(d,k,o) on-chip; output f32r (rounds for the PE)
        wperm = wpermp.tile([P, D, K, O], F32R, tag="wperm")
        eng = nc.vector if nb % 2 == 0 else nc.scalar
        if nb % 2 == 0:
            nc.vector.tensor_copy(
                wperm, wnat.rearrange("n (k d o) -> n d k o", k=K, d=D))
        else:
            nc.scalar.activation(
                wperm, wnat.rearrange("n (k d o) -> n d k o", k=K, d=D),
                ACT_F.Copy)
        # SBUF->SBUF reshuffle into matmul layout [(v,d),(k,o)] per group
        wt = wtp.tile([P, GL, KO], F32R, tag="wt")
        for gi in range(GL):
            nc.gpsimd.dma_start(
                wt[:, gi],
                wperm[ds(gi * V, V)].rearrange("v d k o -> v d (k o)"))
        # block-diagonal lhsT for the batch (f32r)
        LT = ltp.tile([P, GL, B, V], F32R, tag="LT")
        nc.vector.tensor_tensor(
            LT,
            XT2[:, nb][:, :, :, None].to_broadcast((P, GL, B, V)),
            M[:, None].to_broadcast((P, GL, B, V)),
            op=OP.mult)
        for gi in range(GL):
            g = nb * GL + gi
            lhs = LT[:, gi].rearrange("p b v -> p (b v)")
            pe = psum_e.tile([P, KO], F32, tag="pe")
            nc.tensor.matmul(pe, lhsT=lhs, rhs=wt[:, gi], start=True, stop=True)
            # s0 accumulator: sum_g hats in a second psum bank
            nc.tensor.matmul(ps0, lhsT=lhs, rhs=wt[:, gi],
                             start=(g == 0), stop=(g == G - 1))
            if gi % 2 == 0:
                nc.vector.tensor_copy(H[:, g], pe)
            else:
                nc.scalar.activation(H[:, g], pe, ACT_F.Copy)

    # ---------------- s0 tail: mean over v', squash ----------------
    s0red = small.tile([P, KO], F32, tag="s0red")
    nc.vector.tensor_copy(s0red, ps0)
    ps0b = psum_r.tile([P, KO], F32, tag="r")
    nc.tensor.matmul(ps0b[:B], lhsT=ONESB16, rhs=s0red, start=True, stop=True)
    out0 = small.tile([B, KO], F32, tag="out0")
    _squash_bk(nc, small, out0, ps0b[:B])
    nc.sync.dma_start(scr.rearrange("b k o -> b (k o)"), out0)

    # ---------------- bias + sweeps ----------------
    bias = hpool.tile([P, G, K], F32)
    nc.vector.memset(bias, 0.0)

    NCH = G // CHUNK
    for it in range(NUM_ROUTINGS - 1):
        last = it == NUM_ROUTINGS - 2
        # OutB[(b,v'), (k,o)] = out_it[b, k, o] broadcast over v' (bf16)
        OutBf = sweep.tile([P, KO], F32, tag="OutBf")
        nc.sync.dma_start(OutBf, scr[:, None].to_broadcast((B, V, K, O)))
        OutB = sweep.tile([P, K, O], BF16, tag="OutB")
        nc.vector.tensor_copy(OutB, OutBf.rearrange("p (k o) -> p k o", k=K))

        pr1 = psum_r.tile([P, KO], F32, tag="r", name=f"pr1_{it}")
        for j in range(NCH):
            gsl = ds(j * CHUNK, CHUNK)
            # R2: a = sum_o H*OutB ; bias += a   (bf16 4x-mode ops)
            prod = sweep.tile([P, CHUNK, K, O], BF16, tag="prod")
            nc.vector.tensor_tensor(
                prod, H[:, gsl].rearrange("p g (k o) -> p g k o", k=K),
                OutB[:, None].to_broadcast((P, CHUNK, K, O)),
                op=OP.mult)
            ach = sweep.tile([P, CHUNK, K], BF16, tag="ach")
            nc.vector.tensor_reduce(ach, prod, axis=AX.X, op=OP.add)
            nc.vector.tensor_tensor(bias[:, gsl], bias[:, gsl], ach, op=OP.add)
            # softmax over k
            expb = sweep.tile([P, CHUNK, K], BF16, tag="expb")
            nc.scalar.activation(expb, bias[:, gsl], ACT_F.Exp)
            den = sweep.tile([P, CHUNK], F32, tag="den")
            nc.vector.tensor_reduce(den, expb, axis=AX.X, op=OP.add)
            rden = sweep.tile([P, CHUNK], F32, tag="rden")
            nc.vector.reciprocal(rden, den)
            cch = sweep.tile([P, CHUNK, K], BF16, tag="cch")
            nc.vector.tensor_tensor(
                cch, expb, rden[:, :, None].to_broadcast((P, CHUNK, K)),
                op=OP.mult)
            # c-lhsT[(b,v'), (g, b', k')] = c * delta_{b,b'}  (bf16)
            LTc = sweep.tile([P, CHUNK, B, K], BF16, tag="LTc")
            for b in range(B):
                nc.vector.tensor_scalar(LTc[:, :, b], cch, Ib[:, b:b + 1], None,
                                        op0=OP.mult)
            for i in range(CHUNK):
                g = j * CHUNK + i
                nc.tensor.matmul(pr1, lhsT=LTc[:, i].rearrange("p b k -> p (b k)"),
                                 rhs=H[:, g],
                                 start=(g == 0), stop=(g == G - 1))

        # diagonal extract: s[(b,k'), o] = sum_k pr1 * delta_{k,k'}
        prodD = small.tile([P, KO], F32, tag="prodD")
        nc.vector.tensor_tensor(prodD, pr1, MK, op=OP.mult)
        sD = small.tile([P, O], F32, tag="sD")
        nc.vector.tensor_reduce(
            sD, prodD.rearrange("p (k o) -> p o k", k=K), axis=AX.X, op=OP.add)
        outN = small.tile([P, O], F32, tag="outN")
        _squash(nc, small, outN, sD, P)
        # NB: rearrange must live on the DRAM side — an SBUF source AP that
        # splits the partition dim silently reads the wrong partitions.
        nc.sync.dma_start((out_d if last else scr).rearrange("b k o -> (b k) o"),
                          outN)

    ctx.close()


def _squash_bk(nc, pool, out, s_ap):
    """Per-k squash of s_ap [B, (k,o)]: norm over o only, for each k."""
    s_sb = pool.tile([B, K, O], F32, tag="sqk_s")
    nc.vector.tensor_copy(s_sb, s_ap.rearrange("b (k o) -> b k o", k=K))
    sq = pool.tile([B, K, O], F32, tag="sqk_tmp")
    nc.vector.tensor_tensor(sq, s_sb, s_sb, op=OP.mult)
    ss = pool.tile([B, K], F32, tag="sqk_ss")
    nc.vector.tensor_reduce(ss, sq, axis=AX.X, op=OP.add)
    rt = pool.tile([B, K], F32, tag="sqk_rt")
    nc.scalar.activation(rt, ss, ACT_F.Sqrt)
    dn = pool.tile([B, K], F32, tag="sqk_dn")
    nc.vector.tensor_scalar(dn, ss, 1.0, None, op0=OP.add)
    rc = pool.tile([B, K], F32, tag="sqk_rc")
    nc.vector.reciprocal(rc, dn)
    sc = pool.tile([B, K], F32, tag="sqk_sc")
    nc.vector.tensor_tensor(sc, rt, rc, op=OP.mult)
    nc.vector.tensor_tensor(
        out.rearrange("b (k o) -> b k o", k=K), s_sb,
        sc[:, :, None].to_broadcast((B, K, O)), op=OP.mult)


def _squash(nc, pool, out, s_ap, nparts):
    """out = s * sqrt(ss)/(1+ss), ss = sum over free dim of s^2 (per partition)."""
    s_sb = pool.tile([nparts, s_ap.shape[-1]], F32, tag="sq_s")
    nc.vector.tensor_copy(s_sb, s_ap)
    s_ap = s_sb
    # (tensor_tensor_reduce hits an "ISA wrong length" codegen bug here,
    # so square and reduce separately)
    sq = pool.tile([nparts, s_ap.shape[-1]], F32, tag="sq_tmp")
    nc.vector.tensor_tensor(sq, s_ap, s_ap, op=OP.mult)
    ss = pool.tile([nparts, 1], F32, tag="sq_ss")
    nc.vector.tensor_reduce(ss, sq, axis=AX.X, op=OP.add)
    rt = pool.tile([nparts, 1], F32, tag="sq_rt")
    nc.scalar.activation(rt, ss, ACT_F.Sqrt)
    dn = pool.tile([nparts, 1], F32, tag="sq_dn")
    nc.vector.tensor_scalar(dn, ss, 1.0, None, op0=OP.add)
    rc = pool.tile([nparts, 1], F32, tag="sq_rc")
    nc.vector.reciprocal(rc, dn)
    sc = pool.tile([nparts, 1], F32, tag="sq_sc")
    nc.vector.tensor_tensor(sc, rt, rc, op=OP.mult)
    nc.vector.tensor_scalar(out, s_ap, sc, None, op0=OP.mult)


_NC_CACHE = None


def kernel(x: np.ndarray, W: np.ndarray) -> np.ndarray:
    global _NC_CACHE
    x = np.ascontiguousarray(x, dtype=np.float32)
    W = np.ascontiguousarray(W, dtype=np.float32)
    if _NC_CACHE is None:
        _NC_CACHE = build_kernel()
    nc = _NC_CACHE
    n_cores = 8
    bsz = x.shape[0] // n_cores  # 8
    consts = _build_consts()
    in_maps = [{"x": x[c * bsz:(c + 1) * bsz], "w": W, "consts": consts}
               for c in range(n_cores)]
    res = run_bass_kernel_spmd(nc, in_maps, core_ids=list(range(n_cores)))
    return np.concatenate([r["out"] for r in res.results], axis=0)
